# revision 3
# baseline (speedup 1.0000x reference)
"""MultiHeadSelfAttention (B=4, C=256, H=W=64, 4 heads, GroupNorm32) on 8 trn2 cores.

fp16 matmul operands (fp32 PSUM accumulation); scores via row-tiled pairs
(two K=64 matmuls run concurrently in distinct PE row-groups); attention is
paced by the Act engine (softmax exp, FD=1024 per call, ~1.08us) with PE
hidden underneath.  The qkv projection for t8 blocks 2..7 is drip-fed into
attention phase (ti=0, j=0)'s chunk loop so PE does qkv work while scores
wait on the interleave score ring; the remaining 7 phases run on a
triple-buffered score ring.  Softmax-normalize tails are deferred into the
next phase's loop so the slow DVE reciprocal never blocks the PE queue; the
final tail computes recip via Act exp(-ln(d)) instead.

Sharding: core = (batch b, T-half), T axis rolled so each core's 2048
output tokens are the first TH columns.
"""

import numpy as np

import concourse.bass as bass
import concourse.mybir as mybir
import concourse.tile as tile
from concourse.bass_utils import run_bass_kernel_spmd

B, C, HH, WW = 4, 256, 64, 64
T = HH * WW            # 4096
TH = T // 2            # 2048 tokens per core
NH = 4                 # heads
CH = C // NH           # 64 channels per head
NG = 32                # groupnorm groups
GS = C // NG           # 8 channels per group
EPS = 1e-5
SCALE2 = CH ** -0.5    # 1/8, applied inside exp()
N_CORES = 8
NCH = T // 128         # 32 s-chunks

F32 = mybir.dt.float32
F16 = mybir.dt.float16
AF = mybir.ActivationFunctionType
OP = mybir.AluOpType


def split_excess_waits(nc, max_waits=1):
    """This container's walrus accepts at most one sync-wait condition per
    instruction; move extras onto preceding same-engine NOPs."""
    for f in nc.m.functions:
        for blk in f.blocks:
            new_insts = []
            for inst in blk.instructions:
                si = getattr(inst, "sync_info", None)
                if si is not None and si.on_wait and len(si.on_wait) > max_waits:
                    head = list(si.on_wait)
                    k = 0
                    while len(head) > max_waits:
                        chunk, head = head[:max_waits], head[max_waits:]
                        new_insts.append(mybir.InstNoOp(
                            name=f"{inst.name}-ws{k}", engine=inst.engine,
                            ins=[], outs=[],
                            sync_info=mybir.SyncInfo(on_wait=chunk, on_update=[])))
                        k += 1
                    si.on_wait = head
                new_insts.append(inst)
            blk.instructions = new_insts


def build_nc(repeat=1):
    nc = bass.Bass("TRN2", target_bir_lowering=False, debug=False)

    xb = nc.dram_tensor("xb", [2, 128, T], F32, kind="ExternalInput")
    qkvwt = nc.dram_tensor("qkvwt", [2, 128, 3 * C], F32, kind="ExternalInput")
    projwt = nc.dram_tensor("projwt", [2, 128, C], F32, kind="ExternalInput")
    qkvb = nc.dram_tensor("qkvb", [128, 6], F32, kind="ExternalInput")
    projb = nc.dram_tensor("projb", [128, 2], F32, kind="ExternalInput")
    normw = nc.dram_tensor("normw", [128, 2], F32, kind="ExternalInput")
    normb = nc.dram_tensor("normb", [128, 2], F32, kind="ExternalInput")
    gsum = nc.dram_tensor("gsum", [128, 16], F32, kind="ExternalInput")
    gbc = nc.dram_tensor("gbc", [16, 128], F32, kind="ExternalInput")
    out_d = nc.dram_tensor("out", [2, 128, TH], F32, kind="ExternalOutput")

    import contextlib

    with tile.TileContext(nc) as tc:
        with (
            tc.tile_pool(name="consts", bufs=1) as consts,
            tc.tile_pool(name="xpool", bufs=1) as xpool,
            tc.tile_pool(name="kqv", bufs=1) as kqv,
            tc.For_i(0, repeat, 1, staggered_reset=True) if repeat > 1 else contextlib.nullcontext(),
        ):
            # x load first: 4 transfers of [128, 2048] on two DGE queues
            x_sb = xpool.tile([128, 2, T], F32)
            for k in range(2):
                for half in range(2):
                    c0 = 2048 * half
                    eng = nc.sync if (2 * k + half) % 2 == 0 else nc.scalar
                    eng.dma_start(out=x_sb[:, k, c0:c0 + 2048],
                                  in_=xb.ap()[k][:, c0:c0 + 2048])

            # ---- constant loads ----
            qkvb_sb = consts.tile([128, 6], F32)
            nc.sync.dma_start(out=qkvb_sb, in_=qkvb.ap())
            projb_sb = consts.tile([128, 2], F32)
            nc.sync.dma_start(out=projb_sb, in_=projb.ap())
            normw_sb = consts.tile([128, 2], F32)
            nc.sync.dma_start(out=normw_sb, in_=normw.ap())
            normb_sb = consts.tile([128, 2], F32)
            nc.sync.dma_start(out=normb_sb, in_=normb.ap())
            gsum_sb = consts.tile([128, 16], F32)
            nc.sync.dma_start(out=gsum_sb, in_=gsum.ap())
            gbc_sb = consts.tile([16, 128], F32)
            nc.sync.dma_start(out=gbc_sb, in_=gbc.ap())
            qkvwt16 = consts.tile([128, 2, 3 * C], F16)
            projwt16 = consts.tile([128, 2, C], F16)
            ones16 = consts.tile([128, CH], F16)
            nc.vector.memset(ones16, 1.0)
            scale_sb = consts.tile([128, 2], F32)
            bias_sb = consts.tile([128, 2], F32)
            # warm the Ln/Exp ACT table set while the x DMA streams
            warm = consts.tile([1, 2], F32)
            nc.vector.memset(warm, 1.0)
            nc.scalar.activation(out=warm, in_=warm, func=AF.Ln)
            nc.scalar.activation(out=warm, in_=warm, func=AF.Exp)

            # persistent qkv outputs (fp16)
            xn16 = kqv.tile([128, 2, T], F16)
            k_sb = kqv.tile([128, 2, T], F16)
            q_sb = kqv.tile([128, 2, TH], F16)
            vt_sb = kqv.tile([128, NCH, NH, CH + 2], F16)
            a_sb = kqv.tile([128, 2, TH], F16)
            out_sb = kqv.tile([128, 2, TH], F32)
            ones_st = consts.tile([128, NCH, 1], F16)
            nc.vector.memset(ones_st, 1.0)
            for h in range(NH):
                nc.vector.tensor_copy(out=vt_sb[:, :, h, CH:CH + 1], in_=ones_st)

            with (
                tc.tile_pool(name="stage", bufs=1) as stage,
                tc.tile_pool(name="psmall", bufs=1, space="PSUM") as psmall,
            ):
                qkvwt_f = stage.tile([128, 2, 3 * C], F32)
                projwt_f = stage.tile([128, 2, C], F32)
                for k in range(2):
                    nc.scalar.dma_start(out=qkvwt_f[:, k, :], in_=qkvwt.ap()[k])
                    nc.scalar.dma_start(out=projwt_f[:, k, :], in_=projwt.ap()[k])
                nc.vector.tensor_copy(out=qkvwt16, in_=qkvwt_f)
                nc.vector.tensor_copy(out=projwt16, in_=projwt_f)

                # ---- groupnorm statistics ----
                stat = stage.tile([128, 2, 2], F32)
                sq = stage.tile([128, 1], F32)
                for k in range(2):
                    st6 = stage.tile([128, 8, 6], F32, bufs=2)
                    for sub in range(8):
                        nc.vector.bn_stats(out=st6[:, sub, :],
                                           in_=x_sb[:, k, 512 * sub:512 * (sub + 1)])
                    nc.vector.bn_aggr(out=stat[:, k, :], in_=st6)
                    nc.vector.tensor_tensor(out=sq, in0=stat[:, k, 0:1],
                                            in1=stat[:, k, 0:1], op=OP.mult)
                    nc.vector.tensor_tensor(out=stat[:, k, 1:2], in0=stat[:, k, 1:2],
                                            in1=sq, op=OP.add)
                pgrp = psmall.tile([16, 4], F32)
                nc.tensor.matmul(pgrp, gsum_sb,
                                 stat.rearrange("p a b -> p (a b)"),
                                 start=True, stop=True)
                pgrp_kv = pgrp.rearrange("g (k v) -> g v k", v=2)
                meang = stage.tile([16, 2], F32)
                nc.vector.tensor_copy(out=meang, in_=pgrp_kv[:, 0, :])
                sqg = stage.tile([16, 2], F32)
                nc.vector.tensor_tensor(out=sqg, in0=meang, in1=meang, op=OP.mult)
                varg = stage.tile([16, 2], F32)
                nc.vector.tensor_tensor(out=varg, in0=pgrp_kv[:, 1, :], in1=sqg,
                                        op=OP.subtract)
                eps_t = stage.tile([16, 1], F32)
                nc.vector.memset(eps_t, EPS)
                # rstd = exp(-0.5*ln(var+eps)); Ln+Exp share one ACT table set
                lvar = stage.tile([16, 2], F32)
                nc.scalar.activation(out=lvar, in_=varg, func=AF.Ln, bias=eps_t)
                rstdg = stage.tile([16, 2], F32)
                nc.scalar.activation(out=rstdg, in_=lvar, func=AF.Exp, scale=-0.5)
                pm = psmall.tile([128, 2], F32)
                nc.tensor.matmul(pm, gbc_sb, meang, start=True, stop=True)
                pr = psmall.tile([128, 2], F32)
                nc.tensor.matmul(pr, gbc_sb, rstdg, start=True, stop=True)
                nc.vector.tensor_tensor(out=scale_sb, in0=pr, in1=normw_sb,
                                        op=OP.mult)
                nc.vector.tensor_tensor(out=bias_sb, in0=pm, in1=scale_sb,
                                        op=OP.mult)
                nc.vector.tensor_tensor(out=bias_sb, in0=normb_sb, in1=bias_sb,
                                        op=OP.subtract)

            # ---------------- fused qkv + attention ----------------
            with (
                tc.tile_pool(name="wexp", bufs=4) as wexp,
                tc.tile_pool(name="apre", bufs=2) as aprep,
                tc.tile_pool(name="rpool", bufs=2) as rpool,
                tc.tile_pool(name="pav", bufs=1, space="PSUM") as pav,
            ):
                deferred = {}

                def make_tail(ti, j, pa, pssp, last=False):
                    t0 = 512 * ti
                    a_pre = aprep.tile([65, 2, 512], F32, tag="apre")
                    for b in range(2):
                        nc.vector.tensor_copy(out=a_pre[:, b, :], in_=pa[:, b, :])
                    rcp16 = rpool.tile([1, 2, 512], F16, tag="rcp16")
                    if last:
                        # Act idle after the last phase: recip = exp(-ln(d)),
                        # with ln reading the psum rows directly so it runs in
                        # parallel with the a_pre copies on DVE
                        lnd = rpool.tile([1, 2, 512], F32, tag="rcp")
                        nc.scalar.activation(out=lnd, in_=pa[64:65, :, :],
                                             func=AF.Ln)
                        nc.scalar.activation(out=rcp16, in_=lnd, func=AF.Exp,
                                             scale=-1.0)
                    else:
                        rcp = rpool.tile([1, 2, 512], F32, tag="rcp")
                        nc.vector.reciprocal(out=rcp, in_=a_pre[64:65, :, :])
                        nc.vector.tensor_copy(out=rcp16, in_=rcp)

                    def part2():
                        rb = pssp.tile([128, 2, 512], F32, tag="ps")
                        for b in range(2):
                            nc.tensor.matmul(rb[0:CH, b, :], ones16[0:1, :],
                                             rcp16[0:1, b, :], start=True,
                                             stop=True)
                        for b in range(2):
                            a_sl = a_sb[CH * b:CH * (b + 1), j, t0:t0 + 512]
                            nc.vector.tensor_tensor(
                                out=a_sl, in0=a_pre[0:CH, b, :],
                                in1=rb[0:CH, b, :], op=OP.mult)
                            nc.vector.tensor_scalar_add(
                                out=a_sl, in0=a_sl,
                                scalar1=qkvb_sb[CH * b:CH * (b + 1), 4 + j:5 + j])
                    return part2

                def make_proj(ti, pssp):
                    t0 = 512 * ti

                    def emit():
                        ph2 = pssp.tile([128, 2, 512], F32, tag="ps")
                        for jj in range(2):
                            for k in range(2):
                                nc.tensor.matmul(
                                    ph2[:, jj, :],
                                    projwt16[:, k, 128 * jj:128 * (jj + 1)],
                                    a_sb[:, k, t0:t0 + 512], start=(k == 0),
                                    stop=(k == 1))
                        for jj in range(2):
                            o_sl = out_sb[:, jj, t0:t0 + 512]
                            nc.vector.tensor_tensor(out=o_sl, in0=ph2[:, jj, :],
                                                    in1=x_sb[:, jj, t0:t0 + 512],
                                                    op=OP.add)
                            nc.vector.tensor_scalar_add(
                                out=o_sl, in0=o_sl, scalar1=projb_sb[:, jj:jj + 1])
                            nc.sync.dma_start(out=out_d.ap()[jj, :, t0:t0 + 512],
                                              in_=o_sl)
                    return emit

                def emit_scores(j, t0, c, ps_sl):
                    nc.tensor.matmul(
                        ps_sl[:, 0, :], k_sb[0:CH, j, 128 * c:128 * (c + 1)],
                        q_sb[0:CH, j, t0:t0 + 512], start=True, stop=True)
                    nc.tensor.matmul(
                        ps_sl[:, 1, :], k_sb[CH:128, j, 128 * c:128 * (c + 1)],
                        q_sb[CH:128, j, t0:t0 + 512], start=True, stop=True)

                def emit_av(j, pa, cc, w_p):
                    for b in range(2):
                        nc.tensor.matmul(
                            pa[:, b, :], vt_sb[:, cc, 2 * j + b, 0:CH + 1],
                            w_p[:, b, :], start=(cc == 0), stop=(cc == NCH - 1))

                # ---- interleaved scope: qkv for t8 blocks + phase (0,0) ----
                if True:
                    def qkv_quanta(t8, psqk, psv, use_act=False):
                        t0 = 512 * t8
                        ops = []
                        for k in range(2):
                            def xn_op(k=k):
                                nc.vector.tensor_scalar(
                                    out=xn16[:, k, t0:t0 + 512],
                                    in0=x_sb[:, k, t0:t0 + 512],
                                    scalar1=scale_sb[:, k:k + 1],
                                    scalar2=bias_sb[:, k:k + 1],
                                    op0=OP.mult, op1=OP.add)
                            ops.append(xn_op)
                        for j in range(2):
                            def k_op(j=j):
                                pk = psqk.tile([128, 512], F32, tag="qk",
                                               name="pk")
                                for k in range(2):
                                    nc.tensor.matmul(
                                        pk,
                                        qkvwt16[:, k, C + 128 * j:C + 128 * (j + 1)],
                                        xn16[:, k, t0:t0 + 512], start=(k == 0),
                                        stop=(k == 1))
                                if use_act:
                                    nc.scalar.activation(
                                        out=k_sb[:, j, t0:t0 + 512], in_=pk,
                                        func=AF.Identity,
                                        bias=qkvb_sb[:, 2 + j:3 + j])
                                else:
                                    nc.vector.tensor_scalar_add(
                                        out=k_sb[:, j, t0:t0 + 512], in0=pk,
                                        scalar1=qkvb_sb[:, 2 + j:3 + j])
                            ops.append(k_op)
                            if t8 < 4:
                                def q_op(j=j):
                                    pq = psqk.tile([128, 512], F32, tag="qk",
                                                   name="pq")
                                    for k in range(2):
                                        nc.tensor.matmul(
                                            pq,
                                            qkvwt16[:, k, 128 * j:128 * (j + 1)],
                                            xn16[:, k, t0:t0 + 512],
                                            start=(k == 0), stop=(k == 1))
                                    if use_act:
                                        nc.scalar.activation(
                                            out=q_sb[:, j, t0:t0 + 512], in_=pq,
                                            func=AF.Identity,
                                            bias=qkvb_sb[:, j:j + 1])
                                    else:
                                        nc.vector.tensor_scalar_add(
                                            out=q_sb[:, j, t0:t0 + 512], in0=pq,
                                            scalar1=qkvb_sb[:, j:j + 1])
                                ops.append(q_op)
                        for u in range(4):
                            def v_op(u=u):
                                pv = psv.tile([128, C], F32, name="pv")
                                for k in range(2):
                                    nc.tensor.matmul(
                                        pv,
                                        xn16[:, k, t0 + 128 * u:t0 + 128 * (u + 1)],
                                        qkvwt16[:, k, 2 * C:3 * C],
                                        start=(k == 0), stop=(k == 1))
                                if use_act:
                                    nc.scalar.activation(
                                        out=vt_sb[:, 4 * t8 + u, :, 0:CH],
                                        in_=pv.rearrange("p (h c) -> p h c",
                                                         h=NH),
                                        func=AF.Identity)
                                else:
                                    nc.vector.tensor_copy(
                                        out=vt_sb[:, 4 * t8 + u, :, 0:CH],
                                        in_=pv.rearrange("p (h c) -> p h c",
                                                         h=NH))
                            ops.append(v_op)
                        return ops

                    # t8 blocks 0 and 1 up front in their own deeper
                    # psum scope (dense back-to-back emission); 2..7 fused
                    # into the chunk loop (sparse, shallow buffers suffice)
                    with (
                        tc.tile_pool(name="psqk0", bufs=3,
                                     space="PSUM") as psqk0,
                        tc.tile_pool(name="psv0", bufs=2, space="PSUM") as psv0,
                    ):
                        for t8 in range(2):
                            for op in qkv_quanta(t8, psqk0, psv0,
                                                 use_act=True):
                                op()
                    with (
                        tc.tile_pool(name="psi", bufs=2, space="PSUM") as psi,
                        tc.tile_pool(name="psqk", bufs=1, space="PSUM") as psqk,
                        tc.tile_pool(name="psv", bufs=1, space="PSUM") as psv,
                    ):
                        quanta = []
                        for t8 in range(2, 8):
                            quanta.extend(qkv_quanta(t8, psqk, psv))

                        pa = pav.tile([65, 2, 512], F32, tag="pav")
                        qi = 0
                        w_prev = None
                        for c in range(NCH):
                            # drip-feed qkv: even pace, but block t8 must be
                            # fully emitted before chunk 4*t8's scores
                            nq = len(quanta)
                            need = min(nq, max((nq * (c + 1)) // NCH,
                                               10 * max(0, c // 4 - 1)))
                            while qi < need:
                                quanta[qi]()
                                qi += 1
                            ps = psi.tile([128, 2, 512], F32, tag="psi")
                            emit_scores(0, 0, c, ps)
                            w_t = wexp.tile([128, 2, 512], F16, tag="w")
                            nc.scalar.activation(out=w_t, in_=ps, func=AF.Exp,
                                                 scale=SCALE2)
                            if w_prev is not None:
                                emit_av(0, pa, c - 1, w_prev)
                            w_prev = w_t
                        while qi < len(quanta):
                            quanta[qi]()
                            qi += 1
                        emit_av(0, pa, NCH - 1, w_prev)

                # ---- main attention scope: remaining 7 phases ----
                with tc.tile_pool(name="pss", bufs=3, space="PSUM") as pss:
                    deferred[1] = [make_tail(0, 0, pa, pss)]
                    phases = [(ti, j) for ti in range(4) for j in range(2)][1:]
                    LAG = 2
                    for pi, (ti, j) in enumerate(phases, start=1):
                        t0 = 512 * ti
                        pa = pav.tile([65, 2, 512], F32, tag="pav")
                        w_ring = [None] * (LAG + 1)
                        for c in range(NCH + LAG):
                            if c < NCH:
                                ps = pss.tile([128, 2, 512], F32, tag="ps")
                                emit_scores(j, t0, c, ps)
                                w_t = wexp.tile([128, 2, 512], F16, tag="w")
                                nc.scalar.activation(out=w_t, in_=ps, func=AF.Exp,
                                                     scale=SCALE2)
                                w_ring[c % (LAG + 1)] = w_t
                            if c == 8 and pi in deferred:
                                for fn in deferred.pop(pi):
                                    fn()
                            if c >= LAG:
                                cc = c - LAG
                                emit_av(j, pa, cc, w_ring[cc % (LAG + 1)])
                        part2 = make_tail(ti, j, pa, pss,
                                          last=(pi == len(phases)))
                        nxt = deferred.setdefault(pi + 1, [])
                        nxt.append(part2)
                        if j == 1:
                            nxt.append(make_proj(ti, pss))

                    for fns in deferred.values():
                        for fn in fns:
                            fn()

    split_excess_waits(nc)
    return nc


_NC_CACHE = {}


def _get_nc(repeat=1):
    if repeat not in _NC_CACHE:
        _NC_CACHE[repeat] = build_nc(repeat)
    return _NC_CACHE[repeat]


def _shard_inputs(x, norm_w, norm_b, qkv_w, qkv_b, proj_w, proj_b):
    xr = np.ascontiguousarray(x.reshape(B, 2, 128, T).astype(np.float32))
    perm = np.concatenate([
        np.concatenate([np.arange(3 * CH * h + CH * p, 3 * CH * h + CH * (p + 1))
                        for h in range(NH)])
        for p in range(3)])
    qkv_w = np.asarray(qkv_w)[perm]
    qkv_b = np.asarray(qkv_b)[perm]
    qkvwt = np.ascontiguousarray(qkv_w.T.reshape(2, 128, 3 * C).astype(np.float32))
    projwt = np.ascontiguousarray(proj_w.T.reshape(2, 128, C).astype(np.float32))
    qkvb = np.ascontiguousarray(qkv_b.reshape(6, 128).T.astype(np.float32))
    projb = np.ascontiguousarray(proj_b.reshape(2, 128).T.astype(np.float32))
    normw = np.ascontiguousarray(norm_w.reshape(2, 128).T.astype(np.float32))
    normb = np.ascontiguousarray(norm_b.reshape(2, 128).T.astype(np.float32))
    p = np.arange(128)
    gsum = (p[:, None] // 8 == np.arange(16)[None, :]).astype(np.float32) / GS
    gbc = (np.arange(16)[:, None] == p[None, :] // 8).astype(np.float32)

    in_maps = []
    for c in range(N_CORES):
        b, half = c // 2, c % 2
        xc = np.roll(xr[b], -half * TH, axis=2) if half else xr[b]
        in_maps.append({
            "xb": np.ascontiguousarray(xc),
            "qkvwt": qkvwt, "projwt": projwt,
            "qkvb": qkvb, "projb": projb,
            "normw": normw, "normb": normb,
            "gsum": gsum, "gbc": gbc,
        })
    return in_maps


def _assemble(results):
    out = np.empty((B, 2, 128, T), np.float32)
    for c in range(N_CORES):
        b, half = c // 2, c % 2
        out[b, :, :, half * TH:(half + 1) * TH] = results[c]["out"]
    return out.reshape(B, C, HH, WW)


def kernel(x, norm_w, norm_b, qkv_w, qkv_b, proj_w, proj_b):
    nc = _get_nc()
    in_maps = _shard_inputs(x, norm_w, norm_b, qkv_w, qkv_b, proj_w, proj_b)
    res = run_bass_kernel_spmd(nc, in_maps, core_ids=list(range(N_CORES)))
    return _assemble(res.results)
